# revision 23
# baseline (speedup 1.0000x reference)
"""Self-contained distributed GAT kernel for 8 TRN2 NeuronCores (Bass/Tile).

Sharding: nodes (and incident edges, grouped by destination) across the
8 cores; weights replicated; per-layer feature tables exchanged via
AllGather; segment softmax/aggregation local per destination partition
in a rectangular [dst-row x edge-slot] layout filled by indexed DMA
gathers.

v2: edge gathers are routed through 4 OVERLAPPING int16 row windows
(32768 rows each) instead of rigid quarters; edges whose source falls in
a window overlap are balanced across the two candidate lanes to shrink
the rectangular padding. Gathers for PRE groups are issued ahead of the
consuming vector pipeline so the Pool engine streams descriptor
generation back-to-back instead of stalling on the compute chain.

Padded slots gather a reserved phantom "poison" row (s_src = -120) and
vanish in the softmax; aggregation is exp-weighted and normalized once
per group.

kernel(**inputs) takes FULL inputs, returns (logits, probas) float32.
"""
import sys
import numpy as np

for _p in ('/opt/trn_rl_repo', '/root/.axon_site/_ro/trn_rl_repo'):
    if _p not in sys.path:
        sys.path.append(_p)

import concourse.bacc as bacc
from concourse import mybir, masks
from concourse.tile import TileContext
from concourse.bass_utils import run_bass_kernel_spmd
from contextlib import ExitStack

NCORES = 8
NLANES = 4
NQ = 4            # pool-phase quarters


def preprocess(edge_index, batch, N=100000, BLOCKS=98, NGRAPHS=256, GS=2):
    NPAD = NCORES * BLOCKS * 128
    SLAB = NPAD // NCORES          # 12544
    QROWS = NPAD // 4
    GPC = NGRAPHS // NCORES
    NGRP = (BLOCKS + GS - 1) // GS
    WIN = 32768
    LBASE = np.array([0, 2 * SLAB - 7680, 4 * SLAB - 7680, NPAD - WIN])

    loop = np.arange(N, dtype=np.int64)
    src = np.concatenate([loop, np.asarray(edge_index[0], dtype=np.int64)])
    dst = np.concatenate([loop, np.asarray(edge_index[1], dtype=np.int64)])
    batch = np.asarray(batch, dtype=np.int64)
    E = src.shape[0]
    deg = np.bincount(dst, minlength=N)
    outdeg = np.bincount(src, minlength=N)

    # ---- pass 1: cores by total in-degree (snake), cap SLAB-1 ----
    order = np.argsort(-deg, kind='stable')
    node_core = np.empty(N, np.int64)
    blk = np.arange(N) // 128
    s_, j_ = np.divmod(blk, NCORES)
    node_core[order] = np.where(s_ % 2 == 0, j_, NCORES - 1 - j_)
    CAP = SLAB - 1
    counts = np.bincount(node_core, minlength=NCORES)
    for c in range(NCORES):
        while counts[c] > CAP:
            tgt = int(np.argmin(counts))
            nodes_c = np.where(node_core == c)[0]
            mv = nodes_c[np.argmin(deg[nodes_c])]
            node_core[mv] = tgt
            counts[c] -= 1
            counts[tgt] += 1

    # ---- flex regions (fixed before clustering): high-outdeg nodes into
    # the window-overlap slot ranges of cores 1,2,3,5 ----
    region = np.zeros(N, np.int8)      # 0 = head/A, 1 = tail/B
    for c in (1, 3, 5):                # tail = slots 38..97 (7680 rows)
        nodes_c = np.where(node_core == c)[0]
        tail_sel = np.argsort(-outdeg[nodes_c], kind='stable')[:7680 - 1]
        r = np.zeros(len(nodes_c), np.int8)
        r[tail_sel] = 1
        region[nodes_c] = r
    nodes_c = np.where(node_core == 2)[0]   # head = slots 0..59 (7680 rows)
    head_sel = np.argsort(-outdeg[nodes_c], kind='stable')[:7680]
    r = np.ones(len(nodes_c), np.int8)
    r[head_sel] = 0
    region[nodes_c] = r

    lane_lo = np.empty(N, np.int8)
    lane_hi = np.empty(N, np.int8)

    def setl(mask, lo, hi):
        lane_lo[mask] = lo
        lane_hi[mask] = hi

    setl(node_core == 0, 0, 0)
    setl((node_core == 1) & (region == 0), 0, 0)
    setl((node_core == 1) & (region == 1), 0, 1)
    setl((node_core == 2) & (region == 0), 0, 1)
    setl((node_core == 2) & (region == 1), 1, 1)
    setl((node_core == 3) & (region == 0), 1, 1)
    setl((node_core == 3) & (region == 1), 1, 2)
    setl(node_core == 4, 2, 2)
    setl((node_core == 5) & (region == 0), 2, 2)
    setl((node_core == 5) & (region == 1), 2, 3)
    setl(node_core >= 6, 3, 3)

    # ---- per-dst lane balancing over the 4-lane path graph ----
    slo = lane_lo[src]
    shi = lane_hi[src]
    f = np.zeros((N, 4), np.int64)
    fixed = slo == shi
    np.add.at(f, (dst[fixed], slo[fixed]), 1)
    g = np.zeros((N, 3), np.int64)
    np.add.at(g, (dst[~fixed], slo[~fixed]), 1)
    T = np.zeros(N, np.int64)
    for i in range(4):
        for j in range(i, 4):
            tot = f[:, i:j + 1].sum(1)
            for fj in range(3):
                if fj >= i and fj + 1 <= j:
                    tot = tot + g[:, fj]
            T = np.maximum(T, -(-tot // (j - i + 1)))
    x = np.zeros((N, 3), np.int64)
    l0 = f[:, 0].copy()
    x[:, 0] = np.clip(T - l0, 0, g[:, 0])
    l0 += x[:, 0]
    l1 = f[:, 1] + (g[:, 0] - x[:, 0])
    x[:, 1] = np.clip(T - l1, 0, g[:, 1])
    l1 += x[:, 1]
    l2 = f[:, 2] + (g[:, 1] - x[:, 1])
    x[:, 2] = np.clip(T - l2, 0, g[:, 2])
    l2 += x[:, 2]
    l3 = f[:, 3] + (g[:, 2] - x[:, 2])
    assert (l3 <= T).all()
    prof = np.stack([l0, l1, l2, l3], axis=1)
    assert (prof.sum(1) == deg).all()

    lane = np.where(fixed, slo, -1).astype(np.int8)
    for j in range(3):
        m = (~fixed) & (slo == j)
        eidx = np.where(m)[0]
        d = dst[eidx]
        o = np.argsort(d, kind='stable')
        eo = eidx[o]
        do = d[o]
        firstm = np.concatenate([[True], do[1:] != do[:-1]])
        runstart = np.maximum.accumulate(np.where(firstm, np.arange(len(do)), 0))
        rank = np.arange(len(do)) - runstart
        lane[eo] = np.where(rank < x[do, j], j, j + 1)
    assert (lane >= 0).all()

    # ---- pass 2: per-core, per-region profile clustering -> (slot,row) ----
    mx = prof.max(axis=1)
    am = prof.argmax(axis=1)
    M = 64
    key = (((((mx * 4 + am) * M + prof[:, 0]) * M + prof[:, 1]) * M
            + prof[:, 2]) * M + prof[:, 3])
    tpos = np.full(N, -1, np.int64)
    region_slots = {
        0: [(0, 0, 98)], 4: [(0, 0, 98)], 6: [(0, 0, 98)], 7: [(0, 0, 98)],
        1: [(0, 0, 38), (1, 38, 98)],
        3: [(0, 0, 38), (1, 38, 98)],
        5: [(0, 0, 38), (1, 38, 98)],
        2: [(0, 0, 60), (1, 60, 98)],
    }
    for c in range(NCORES):
        for (rid, sl, sh) in region_slots[c]:
            nodes_cr = np.where((node_core == c) & (region == rid))[0]
            capr = (sh - sl) * 128 - (1 if sh == 98 else 0)
            assert len(nodes_cr) <= capr, (c, rid, len(nodes_cr), capr)
            o2 = nodes_cr[np.argsort(-key[nodes_cr], kind='stable')]
            rr = np.arange(len(o2))
            tpos[o2] = c * SLAB + (sl + rr // 128) * 128 + rr % 128
    assert (tpos >= 0).all()
    occ = np.zeros(NPAD, bool)
    occ[tpos] = True
    POISON = []
    for k in range(NLANES):
        # phantom rows: last slab row of cores 0,2,4,6 lie inside lanes 0-3
        p = (2 * k + 1) * SLAB - 1
        assert not occ[p] and LBASE[k] <= p < LBASE[k] + WIN
        POISON.append(int(p - LBASE[k]))
    for c in range(NCORES):
        assert not occ[c * SLAB + SLAB - 1], c

    # ---- geometry ----
    d_t = tpos[dst]
    d_core = d_t // SLAB
    d_rem = d_t - d_core * SLAB
    d_slot = d_rem // 128
    d_row = d_rem - d_slot * 128
    s_t = tpos[src]
    s_rel = (s_t - LBASE[lane]).astype(np.int64)
    assert (s_rel >= 0).all() and (s_rel < WIN).all()

    cnt = np.zeros((NCORES, BLOCKS, 128, NLANES), np.int64)
    np.add.at(cnt, (d_core, d_slot, d_row, lane), 1)
    slot_l_max = cnt.max(axis=(0, 2))                    # [BLOCKS, NLANES]

    # one group per slot (so runtime per-core widths can truncate each gather
    # call); widths rounded even for the pair-add reduction
    def even_w(w):
        w = np.maximum(w, 2)
        return w + (w % 2)

    NGRP = BLOCKS
    GSL = np.ones(NGRP, np.int64)
    WG = even_w(slot_l_max)                            # [BLOCKS, NLANES]
    # per-core runtime call widths (even, <= shared width)
    Wcore = even_w(cnt.max(axis=2))                    # [NCORES, BLOCKS, NLANES]
    assert (Wcore <= WG[None]).all()
    wcnt = (128 * Wcore).reshape(NCORES, BLOCKS * NLANES).astype(np.int32)
    d_grp = d_slot
    d_sloc = np.zeros_like(d_slot)
    qg0 = np.zeros((NGRP, NLANES + 1), np.int64)
    for gi in range(NGRP):
        qg0[gi, 1:] = np.cumsum(WG[gi] * GSL[gi])
    GW = qg0[:, -1]
    g0 = np.concatenate([[0], np.cumsum(GW)])
    WTOT = int(g0[-1])

    # rank within (core, slot, row, lane)
    kk = ((d_core * BLOCKS + d_slot) * 128 + d_row) * NLANES + lane
    eorder = np.argsort(kk, kind='stable')
    ks = kk[eorder]
    first = np.concatenate([[True], ks[1:] != ks[:-1]])
    runstart = np.maximum.accumulate(np.where(first, np.arange(E), 0))
    rank = np.arange(E) - runstart

    lidx = np.zeros((NCORES, 128, WTOT), np.int16)
    for gi in range(NGRP):
        for k in range(NLANES):
            c0 = int(g0[gi] + qg0[gi][k])
            c1 = c0 + int(GSL[gi] * WG[gi][k])
            lidx[:, :, c0:c1] = POISON[k]
    padmask = np.ones((NCORES, 128, WTOT), bool)
    eo = eorder
    lane_eo = lane[eo].astype(np.int64)
    col = (g0[d_grp[eo]] + qg0[d_grp[eo], lane_eo]
           + d_sloc[eo] * WG[d_grp[eo], lane_eo] + rank)
    lidx[d_core[eo], d_row[eo], col] = s_rel[eo].astype(np.int16)
    padmask[d_core[eo], d_row[eo], col] = False
    # per-core runtime truncation contract: idx = -1 beyond the core's own
    # width in each (slot, lane) region; num_idxs_reg counts the valid prefix
    for c in range(NCORES):
        for s in range(BLOCKS):
            for k in range(NLANES):
                c0 = int(g0[s] + qg0[s][k])
                lidx[c, :, c0 + int(Wcore[c, s, k]):c0 + int(WG[s, k])] = -1
    maskneg = np.where(padmask, np.float32(-200.0), np.float32(0.0))
    assert (~padmask).sum() == E

    node_at = np.full((NCORES, SLAB), -1, np.int64)
    lp = tpos - (tpos // SLAB) * SLAB
    node_at[tpos // SLAB, lp] = np.arange(N)

    # ---- pooling: graph g -> (core, grow); nodes on partitions
    # 4*grow + subrow (subrow = within-(graph,quarter) rank % 4) ----
    gsize = np.bincount(batch, minlength=NGRAPHS)
    gorder = np.argsort(-gsize, kind='stable')
    pool_core = np.empty(NGRAPHS, np.int64)
    pool_row = np.empty(NGRAPHS, np.int64)
    for i, gg in enumerate(gorder):
        rr, j = divmod(i, NCORES)
        pool_core[gg] = j if rr % 2 == 0 else NCORES - 1 - j
        pool_row[gg] = rr

    keyp = (pool_core[batch] * GPC + pool_row[batch]) * NQ + (tpos // QROWS)
    porder = np.argsort(keyp, kind='stable')
    kp = keyp[porder]
    firstp = np.concatenate([[True], kp[1:] != kp[:-1]])
    runstart = np.maximum.accumulate(np.where(firstp, np.arange(N), 0))
    rankp = np.arange(N) - runstart
    subrow = rankp % 4
    jcol = rankp // 4
    pq = np.zeros((NCORES, GPC, NQ), np.int64)
    np.add.at(pq, (pool_core[batch], pool_row[batch], tpos // QROWS), 1)
    PWQS = np.maximum((pq + 3) // 4, 1).max(axis=(0, 1))   # [NQ]
    pq0 = np.concatenate([[0], np.cumsum(PWQS)])
    WPS = int(pq0[-1])

    pool_lidx = np.zeros((NCORES, 128, WPS), np.int16)
    pool_pad = np.ones((NCORES, 128, WPS), bool)
    pc = pool_core[batch][porder]
    pr = pool_row[batch][porder]
    ppart = pr * 4 + subrow
    pcol = pq0[(tpos // QROWS)[porder]] + jcol
    pool_lidx[pc, ppart, pcol] = (tpos - (tpos // QROWS) * QROWS)[porder].astype(np.int16)
    pool_pad[pc, ppart, pcol] = False
    pool_maskneg = np.where(pool_pad, np.float32(-1e30), np.float32(0.0))
    out_graph = np.empty((NCORES, GPC), np.int64)
    out_graph[pool_core, pool_row] = np.arange(NGRAPHS)

    return dict(
        tpos=tpos, node_at=node_at, BLOCKS=BLOCKS, LBASE=LBASE, WIN=WIN,
        lidx=lidx, maskneg=maskneg, WG=WG, GSL=GSL, qg0=qg0, g0=g0, WTOT=WTOT,
        pool_lidx=pool_lidx, pool_maskneg=pool_maskneg, PWQS=PWQS, pq0=pq0,
        WPS=WPS, out_graph=out_graph, deg=deg, wcnt=wcnt,
    )


def wrap_idx(vals):
    """[..., n] int16, n % 16 == 0: idx i -> [i%16, i//16], replicated x8 to
    128 partitions -> [..., 128, n/16]."""
    sh = vals.shape[:-1]
    n = vals.shape[-1]
    assert n % 16 == 0
    w = vals.reshape(*sh, n // 16, 16)
    w = np.swapaxes(w, -1, -2)
    w = np.broadcast_to(w[..., None, :, :], (*sh, 8, 16, n // 16))
    return w.reshape(*sh, 128, n // 16).copy()


def expand_a(a):
    heads, ch = a.shape
    A = np.zeros((heads * ch, heads), np.float32)
    for h in range(heads):
        A[h * ch:(h + 1) * ch, h] = a[h]
    return A


FP = mybir.dt.float32
BF = mybir.dt.bfloat16
I16 = mybir.dt.int16
ALU = mybir.AluOpType
ACTF = mybir.ActivationFunctionType
AX = mybir.AxisListType

NL = 4
GPC = 32
NEG = 0.2
NEG_OUT = 0.01
HEADS = (4, 4, 4, 1)
PRE = 5          # gather prefetch depth (groups)
USE_WREG = True


def build(nc, geom):
    BLOCKS = int(geom["BLOCKS"])
    NP_ = NCORES * BLOCKS * 128
    SLAB = NP_ // NCORES
    QROWS = NP_ // NQ
    WIN = int(geom["WIN"])
    LBASE = np.asarray(geom["LBASE"])
    WG = np.asarray(geom["WG"])            # [NGRP, NLANES]
    GSL = np.asarray(geom["GSL"])          # [NGRP]
    qg0 = np.asarray(geom["qg0"])          # [NGRP, NLANES+1]
    g0 = np.asarray(geom["g0"])            # [NGRP+1]
    WTOT = int(geom["WTOT"])
    PWQS = np.asarray(geom["PWQS"])        # [NQ]
    pq0 = np.asarray(geom["pq0"])          # [NQ+1]
    WPS = int(geom["WPS"])
    NGRP = len(GSL)

    # ---------------- I/O ----------------
    xT = nc.declare_dram_parameter("xT", [128, SLAB], FP, isOutput=False)
    idx_in = nc.declare_dram_parameter("idx", [128, 8 * WTOT], I16, isOutput=False)
    mask_in = nc.declare_dram_parameter("maskneg", [128, WTOT], FP, isOutput=False)
    wcnt_in = nc.declare_dram_parameter("wcnt", [1, BLOCKS * 4], mybir.dt.int32,
                                        isOutput=False)
    pidx_in = nc.declare_dram_parameter("pool_idx", [128, 8 * WPS], I16, isOutput=False)
    pmask_in = nc.declare_dram_parameter("pool_maskneg", [128, WPS], FP, isOutput=False)
    wext_in = [
        nc.declare_dram_parameter(f"wext{l}", [128 if l == 0 else 64, 64 + 2 * HEADS[l]],
                                  FP, isOutput=False)
        for l in range(NL)
    ]
    bias_in = nc.declare_dram_parameter("bias", [128, NL, 64], FP, isOutput=False)
    fcW_in = nc.declare_dram_parameter("fcW", [64, 2], FP, isOutput=False)
    fcb_in = nc.declare_dram_parameter("fcb", [GPC, 2], FP, isOutput=False)
    logits_out = nc.declare_dram_parameter("logits", [GPC, 2], FP, isOutput=True)
    probas_out = nc.declare_dram_parameter("probas", [GPC, 2], FP, isOutput=True)

    with TileContext(nc) as tc, ExitStack() as ex:
        dram = ex.enter_context(tc.tile_pool(name="dram", bufs=1, space="DRAM"))
        tables = [dram.tile([NP_, 128], I16, addr_space="Shared", name=f"table{l}")
                  for l in range(NL + 1)]
        slabs = [dram.tile([SLAB, 128], I16, name=f"slab{l}") for l in range(NL + 1)]

        cpool = ex.enter_context(tc.tile_pool(name="const", bufs=1))
        gpools = [ex.enter_context(tc.tile_pool(name=f"gath{k}", bufs=PRE + 1))
                  for k in range(NLANES)]
        wpool = ex.enter_context(tc.tile_pool(name="wrk", bufs=3))
        xpool = ex.enter_context(tc.tile_pool(name="xin", bufs=3))
        rpool = ex.enter_context(tc.tile_pool(name="rows", bufs=3))
        ppool = ex.enter_context(tc.tile_pool(name="psum", bufs=4, space="PSUM"))
        ppool2 = ex.enter_context(tc.tile_pool(name="psum2", bufs=2, space="PSUM"))
        ipool = ex.enter_context(tc.tile_pool(name="idxs", bufs=PRE + 1))

        # ---- constants resident in SBUF ----
        wext = []
        for l in range(NL):
            t = cpool.tile([128 if l == 0 else 64, 64 + 2 * HEADS[l]], FP,
                           name=f"wext_sb{l}")
            nc.sync.dma_start(t[:], wext_in[l][:])
            wext.append(t)
        bias_sb = cpool.tile([128, NL, 64], FP)
        nc.sync.dma_start(bias_sb[:], bias_in[:])
        ident = cpool.tile([128, 128], FP)
        masks.make_identity(nc, ident[:])
        poison_t = cpool.tile([1, 4], FP)
        nc.vector.memset(poison_t[:], -120.0)
        wcnt_sb = cpool.tile([1, BLOCKS * 4], mybir.dt.int32)
        nc.sync.dma_start(wcnt_sb[:], wcnt_in[:])
        wreg = nc.gpsimd.alloc_register("gather_wcnt")
        # warm every gather buffer so runtime-truncated gathers leave only
        # bounded stale bf16 data in the masked tail columns
        WMAX = [int(WG[:, k].max()) for k in range(NLANES)]
        for k in range(NLANES):
            for b in range(PRE + 1):
                t = gpools[k].tile([128, WMAX[k], 128], I16, tag=f"G{k}",
                                   name=f"warm_{k}_{b}")
                nc.vector.memset(t[:], 0)
        sdst_self = [cpool.tile([128, BLOCKS, 4], FP, name=f"sdst{i}") for i in range(2)]

        def matmul_to_row(l, s, lhsT_ap):
            """h_ext = lhsT.T @ wext[l] -> row [128,128] bf16 -> slab[l]; also
            stashes s_dst into sdst_self[l % 2]."""
            H = HEADS[l]
            pm = ppool.tile([128, 64 + 2 * H], FP, tag="mm", name=f"mm_{l}_{s}")
            nc.tensor.matmul(pm[:], lhsT_ap, wext[l][:], start=True, stop=True)
            row = rpool.tile([128, 128], I16, tag="row", name=f"row_{l}_{s}")
            nc.scalar.copy(row[:].bitcast(BF)[:, 0:64], pm[:, 0:64])
            rf = row[:].bitcast(FP)
            nc.vector.tensor_copy(rf[:, 32:32 + 2 * H], pm[:, 64:64 + 2 * H])
            # s_dst from the SBUF row (avoids a second PSUM read per block)
            nc.vector.tensor_copy(sdst_self[l % 2][:, s, 0:H], rf[:, 32 + H:32 + 2 * H])
            nc.vector.memset(rf[:, 32 + 2 * H:64], 0.0)
            nc.sync.dma_start(slabs[l][s * 128:(s + 1) * 128, :], row[:])
            if s == BLOCKS - 1:
                # poison row: phantom last slab row's s_src <- -120 so padded
                # gather slots vanish in the softmax (exp(leaky(-120+sdst))~0)
                nc.sync.dma_start(slabs[l][SLAB - 1:SLAB, 64:72],
                                  poison_t[:].bitcast(I16))

        # ---- layer-0 matmul phase ----
        for s in range(BLOCKS):
            xt = xpool.tile([128, 128], FP, tag="xt", name=f"xt0_{s}")
            nc.sync.dma_start(xt[:], xT[:, s * 128:(s + 1) * 128])
            matmul_to_row(0, s, xt[:])

        # ---- layers ----
        for l in range(NL):
            nc.gpsimd.collective_compute(
                "AllGather", ALU.bypass,
                ins=[slabs[l][:].opt()],
                outs=[tables[l][:].opt()],
                replica_groups=[list(range(NCORES))],
            )
            H = HEADS[l]
            ch = 64 // H

            def emit_gather(g):
                GWg = int(qg0[g][NLANES])
                itile = ipool.tile([128, 8 * GWg], I16, tag="idx",
                                   name=f"idx_{l}_{g}")
                nc.sync.dma_start(itile[:], idx_in[:, 8 * int(g0[g]):
                                               8 * int(g0[g] + GWg)])
                mtile = ipool.tile([128, GWg], FP, tag="msk",
                                   name=f"msk_{l}_{g}")
                nc.sync.dma_start(mtile[:], mask_in[:, int(g0[g]):
                                                int(g0[g] + GWg)])
                nsl = int(GSL[g])
                Gq = []
                for k in range(NLANES):
                    wq = int(WG[g][k])
                    ncols = nsl * wq
                    t = gpools[k].tile([128, ncols, 128], I16, tag=f"G{k}",
                                       name=f"G_{l}_{g}_{k}")
                    r0 = int(qg0[g][k])
                    if USE_WREG:
                        nc.gpsimd.reg_load(wreg,
                                           wcnt_sb[0:1, g * 4 + k:g * 4 + k + 1])
                    nc.gpsimd.dma_gather(
                        t[:], tables[l][LBASE[k]:LBASE[k] + WIN, :],
                        itile[:, 8 * r0: 8 * (r0 + ncols)],
                        128 * ncols, wreg if USE_WREG else 128 * ncols, 128,
                        single_packet=False, queue_num=k)
                    Gq.append(t)
                return (Gq, mtile)

            def emit_compute(g, Gqm):
                Gq, mtile = Gqm
                lo = int(sum(GSL[:g]))
                nsl = int(GSL[g])
                den = wpool.tile([128, nsl, 4], FP, tag="den", name=f"den_{l}_{g}")
                outg = wpool.tile([128, nsl, 64], FP, tag="outg", name=f"og_{l}_{g}")
                for k in range(NLANES):
                    wq = int(WG[g][k])
                    Gf = Gq[k][:].bitcast(FP).rearrange("p (s j) e -> p s j e", s=nsl)
                    ssrc = Gf[:, :, :, 32:32 + H]
                    e = wpool.tile([128, nsl, wq, H], FP, tag=f"e{k}",
                                   name=f"e_{l}_{g}_{k}")
                    nc.vector.tensor_tensor(
                        e[:], ssrc,
                        sdst_self[l % 2][:, lo:lo + nsl, 0:H]
                        .unsqueeze(2).broadcast_to([128, nsl, wq, H]),
                        ALU.add)
                    # kill padded/stale columns before exp (mask = -1e4)
                    r0 = int(qg0[g][k])
                    nc.vector.tensor_tensor(
                        e[:], e[:],
                        mtile[:, r0:r0 + nsl * wq]
                        .rearrange("p (s j) -> p s j", s=nsl).unsqueeze(3)
                        .broadcast_to([128, nsl, wq, H]),
                        ALU.add)
                    nc.vector.scalar_tensor_tensor(e[:], e[:], NEG, e[:],
                                                   ALU.mult, ALU.max)
                    ext = wpool.tile([128, nsl, wq, H], BF, tag=f"ex{k}",
                                     name=f"ex_{l}_{g}_{k}")
                    nc.scalar.activation(ext[:], e[:], ACTF.Exp)
                    dq = wpool.tile([128, nsl, 4], FP, tag=f"dq{k}",
                                    name=f"dq_{l}_{g}_{k}")
                    nc.vector.tensor_reduce(
                        dq[:, :, 0:H], ext[:].rearrange("p s j h -> p s h j"),
                        axis=AX.X, op=ALU.add)
                    if k == 0:
                        nc.vector.tensor_copy(den[:, :, 0:H], dq[:, :, 0:H])
                    else:
                        nc.vector.tensor_tensor(den[:, :, 0:H], den[:, :, 0:H],
                                                dq[:, :, 0:H], ALU.add)
                    # unnormalized weighted aggregation (releases Gq early);
                    # stt instead of tensor_tensor for the 2x_2p fast path
                    wt = wpool.tile([128, nsl, wq, 64], BF, tag="wt",
                                    name=f"wt_{l}_{g}_{k}")
                    Gb = Gq[k][:].bitcast(BF).rearrange("p (s j) e -> p s j e",
                                                        s=nsl)
                    for h in range(H):
                        nc.vector.scalar_tensor_tensor(
                            wt[:, :, :, h * ch:(h + 1) * ch],
                            Gb[:, :, :, h * ch:(h + 1) * ch], 0.0,
                            ext[:, :, :, h].unsqueeze(3)
                            .broadcast_to([128, nsl, wq, ch]),
                            ALU.bypass, ALU.mult)
                    # pairwise fold over j (stt, 2x) then a half-width reduce
                    wp = wpool.tile([128, nsl, wq // 2, 64], FP, tag="wp",
                                    name=f"wp_{l}_{g}_{k}")
                    nc.vector.scalar_tensor_tensor(
                        wp[:], wt[:, :, 0::2, :], 0.0, wt[:, :, 1::2, :],
                        ALU.bypass, ALU.add)
                    if k == 0:
                        nc.vector.tensor_reduce(
                            outg[:], wp[:].rearrange("p s j f -> p s f j"),
                            axis=AX.X, op=ALU.add)
                    else:
                        wr = wpool.tile([128, nsl, 64], FP, tag="wr",
                                        name=f"wr_{l}_{g}_{k}")
                        nc.vector.tensor_reduce(
                            wr[:], wp[:].rearrange("p s j f -> p s f j"),
                            axis=AX.X, op=ALU.add)
                        nc.vector.tensor_tensor(outg[:], outg[:], wr[:], ALU.add)
                rden = wpool.tile([128, nsl, 4], FP, tag="rden", name=f"rd_{l}_{g}")
                nc.vector.reciprocal(rden[:, :, 0:H], den[:, :, 0:H])
                nc.vector.tensor_tensor(
                    outg[:].rearrange("p s (h c) -> p s h c", h=H),
                    outg[:].rearrange("p s (h c) -> p s h c", h=H),
                    rden[:, :, 0:H].unsqueeze(3).broadcast_to([128, nsl, H, ch]),
                    ALU.mult)
                # bias + outer leaky for the whole group
                nc.vector.tensor_tensor(
                    outg[:], outg[:],
                    bias_sb[:, l, :].unsqueeze(1).broadcast_to([128, nsl, 64]),
                    ALU.add)
                nc.vector.scalar_tensor_tensor(outg[:], outg[:], NEG_OUT, outg[:],
                                               ALU.mult, ALU.max)
                for si in range(nsl):
                    s = lo + si
                    if l < NL - 1:
                        pt = ppool2.tile([64, 128], FP, tag="tp", name=f"tp_{l}_{s}")
                        nc.tensor.transpose(pt[:], outg[:, si, :], ident[:])
                        xtn = xpool.tile([64, 128], FP, tag="xtn", name=f"xtn_{l}_{s}")
                        nc.scalar.copy(xtn[:], pt[:])
                        matmul_to_row(l + 1, s, xtn[:])
                    else:
                        row = rpool.tile([128, 128], I16, tag="row", name=f"rowF_{s}")
                        rf = row[:].bitcast(FP)
                        nc.vector.tensor_copy(rf[:], outg[:, si, :])
                        nc.sync.dma_start(slabs[NL][s * 128:(s + 1) * 128, :], row[:])

            # software pipeline: gathers run PRE groups ahead of the consumers
            pending = []
            for g in range(NGRP):
                pending.append(emit_gather(g))
                if g >= PRE:
                    emit_compute(g - PRE, pending[g - PRE])
                    pending[g - PRE] = None
            for g in range(max(NGRP - PRE, 0), NGRP):
                emit_compute(g, pending[g])
                pending[g] = None

        # ---- final AllGather (x_final fp32 rows) ----
        nc.gpsimd.collective_compute(
            "AllGather", ALU.bypass,
            ins=[slabs[NL][:].opt()],
            outs=[tables[NL][:].opt()],
            replica_groups=[list(range(NCORES))],
        )

        # ---- pooling ----
        pidx = cpool.tile([128, 8 * WPS], I16)
        nc.sync.dma_start(pidx[:], pidx_in[:])
        pmask = cpool.tile([128, WPS], FP)
        nc.sync.dma_start(pmask[:], pmask_in[:])
        pooled = cpool.tile([128, 64], FP)
        first = True
        PCH = 16
        for q in range(NQ):
            for k0 in range(0, int(PWQS[q]), PCH):
                wq = min(PCH, int(PWQS[q]) - k0)
                c0 = int(pq0[q]) + k0
                PG = gpools[q].tile([128, wq, 128], I16, tag=f"G{q}",
                                    name=f"PG_{q}_{k0}")
                nc.gpsimd.dma_gather(
                    PG[:], tables[NL][q * QROWS:(q + 1) * QROWS, :],
                    pidx[:, 8 * c0: 8 * (c0 + wq)],
                    128 * wq, 128 * wq, 128,
                    single_packet=False, queue_num=q)
                PGf = PG[:].bitcast(FP)                    # [128, wq, 64]
                pm = wpool.tile([128, wq, 64], FP, tag="pm", name=f"pm_{q}_{k0}")
                nc.vector.tensor_tensor(
                    pm[:], PGf,
                    pmask[:, c0:c0 + wq].unsqueeze(2).broadcast_to([128, wq, 64]),
                    ALU.add)
                red = wpool.tile([128, 64], FP, tag="red", name=f"red_{q}_{k0}")
                nc.vector.tensor_reduce(red[:], pm[:].rearrange("p w f -> p f w"),
                                        axis=AX.X, op=ALU.max)
                if first:
                    nc.vector.tensor_copy(pooled[:], red[:])
                    first = False
                else:
                    nc.vector.tensor_tensor(pooled[:], pooled[:], red[:], ALU.max)
        # transpose + fold the 4 subrows per graph via strided max-reduce
        ptp = ppool2.tile([64, 128], FP, tag="tp", name="pool_tp")
        nc.tensor.transpose(ptp[:], pooled[:], ident[:])
        ptps = cpool.tile([64, 128], FP)
        nc.scalar.copy(ptps[:], ptp[:])
        pooledT = cpool.tile([64, GPC], FP)
        nc.vector.tensor_reduce(
            pooledT[:], ptps[:].rearrange("p (g r) -> p g r", r=4),
            axis=AX.X, op=ALU.max)
        # FC + bias + softmax
        fcW = cpool.tile([64, 2], FP)
        nc.sync.dma_start(fcW[:], fcW_in[:])
        fcb = cpool.tile([GPC, 2], FP)
        nc.sync.dma_start(fcb[:], fcb_in[:])
        plog = ppool.tile([GPC, 2], FP, tag="mm", name="logits_mm")
        nc.tensor.matmul(plog[:], pooledT[:], fcW[:], start=True, stop=True)
        logits = cpool.tile([GPC, 2], FP)
        nc.vector.tensor_tensor(logits[:], plog[:], fcb[:], ALU.add)
        nc.sync.dma_start(logits_out[:], logits[:])
        m = cpool.tile([GPC, 1], FP)
        nc.vector.tensor_reduce(m[:], logits[:], axis=AX.X, op=ALU.max)
        z = cpool.tile([GPC, 2], FP)
        nc.vector.tensor_tensor(z[:], logits[:], m[:].broadcast_to([GPC, 2]),
                                ALU.subtract)
        ez = cpool.tile([GPC, 2], FP)
        nc.scalar.activation(ez[:], z[:], ACTF.Exp)
        den2 = cpool.tile([GPC, 1], FP)
        nc.vector.tensor_reduce(den2[:], ez[:], axis=AX.X, op=ALU.add)
        rden2 = cpool.tile([GPC, 1], FP)
        nc.vector.reciprocal(rden2[:], den2[:])
        probas = cpool.tile([GPC, 2], FP)
        nc.vector.tensor_tensor(probas[:], ez[:], rden2[:].broadcast_to([GPC, 2]),
                                ALU.mult)
        nc.sync.dma_start(probas_out[:], probas[:])
    return nc


def make_inputs(P, inp):
    """Per-core in_maps from preprocess() result P and problem inputs."""
    BLOCKS = int(P["BLOCKS"])
    SLAB = BLOCKS * 128
    x = np.asarray(inp["x"], np.float32)
    F = x.shape[1]
    wext_np = []
    for l in range(NL):
        Wl = np.asarray(inp[f"W{l+1}"], np.float32)
        As = expand_a(np.asarray(inp[f"a{l+1}s"], np.float32))
        Ad = expand_a(np.asarray(inp[f"a{l+1}d"], np.float32))
        wext_np.append(np.concatenate([Wl, Wl @ As, Wl @ Ad], axis=1))
    bias_np = np.stack([np.asarray(inp[f"b{l+1}"], np.float32) for l in range(NL)])
    bias_rep = np.tile(bias_np[None], (128, 1, 1))
    fcW = np.asarray(inp["fcW"], np.float32)
    fcb = np.tile(np.asarray(inp["fcb"], np.float32)[None, :], (GPC, 1))

    # wrapped idx: per gather region (column range), stream = col-major
    WG, GSL, qg0, g0 = P["WG"], P["GSL"], P["qg0"], P["g0"]
    NGRP = len(GSL)
    regions = []
    for g in range(NGRP):
        for k in range(NLANES):
            c0 = int(g0[g] + qg0[g][k])
            regions.append((c0, int(GSL[g]) * int(WG[g][k])))
    pregions = [(int(P["pq0"][q]), int(P["PWQS"][q])) for q in range(NQ)]

    def build_idx(lidx_c, regs):
        parts = []
        for c0, ncols in regs:
            stream = lidx_c[:, c0:c0 + ncols].T.reshape(1, -1)   # col-major
            parts.append(wrap_idx(stream)[0])
        return np.concatenate(parts, axis=1).astype(np.int16)

    def build_mask(mask_c, regs):
        parts = [mask_c[:, c0:c0 + ncols] for c0, ncols in regs]
        return np.concatenate(parts, axis=1).astype(np.float32)

    in_maps = []
    for c in range(NCORES):
        nodes = P["node_at"][c]
        xs = np.zeros((SLAB, F), np.float32)
        valid = nodes >= 0
        xs[valid] = x[nodes[valid]]
        m = {
            "xT": np.ascontiguousarray(xs.T),
            "idx": build_idx(P["lidx"][c], regions),
            "maskneg": build_mask(P["maskneg"][c], regions),
            "wcnt": P["wcnt"][c:c + 1],
            "pool_idx": build_idx(P["pool_lidx"][c], pregions),
            "pool_maskneg": P["pool_maskneg"][c].astype(np.float32),
            "bias": bias_rep, "fcW": fcW, "fcb": fcb,
        }
        for l in range(NL):
            m[f"wext{l}"] = wext_np[l]
        in_maps.append(m)
    return in_maps


def _run(inputs, trace=False):
    inp = {k: np.asarray(v) for k, v in inputs.items()}
    P = preprocess(inp['edge_index'], inp['batch'], N=100000, BLOCKS=98,
                   NGRAPHS=256, GS=2)
    in_maps = make_inputs(P, inp)
    nc = bacc.Bacc("TRN2", num_swdge_queues=4, dynamic_dma_scratch_size=16384)
    build(nc, P)
    nc.compile()
    res = run_bass_kernel_spmd(nc, in_maps, list(range(NCORES)), trace=trace)
    logits = np.zeros((256, 2), np.float32)
    probas = np.zeros((256, 2), np.float32)
    for c in range(NCORES):
        lg = res.results[c]["logits"]
        pb = res.results[c]["probas"]
        for r in range(GPC):
            g = P["out_graph"][c, r]
            logits[g] = lg[r]
            probas[g] = pb[r]
    return logits, probas, res.exec_time_ns


def kernel(**inputs):
    logits, probas, _ = _run(inputs, trace=False)
    return logits, probas
